# revision 9
# baseline (speedup 1.0000x reference)
"""Distributed CL loss kernel for Trainium2 (8 NeuronCores).

Reference computes  mean_i sum_j ||s_i - t_j||^2 * [tg_i == tg_j] / cnt[tg_i]
with the [N, N] pairwise-distance matrix.  Because the mask only depends on
the class labels, the whole loss collapses to per-class aggregates:

  loss = (1/N) * [ sum|s|^2 + sum|t|^2 - 2 * sum_c S_c.T_c / cnt_c ]

with S_c / T_c the class-sums of fm_s / fm_t rows.  Device work per core
(rows sharded 512 s-rows + 512 t-rows, fp8e4m3, one-hot cols appended):

  * inputs stream as FOUR 2-tile chunks on the sync HWDGE ring.  The DRAM
    buffer is partition-major [128, 8*1040] so every chunk is one contiguous
    2080 B descriptor per partition (vs 8 x 1040 B in the per-tile layout);
    a single ring is drained by all 16 SDMA engines at full rate and the
    issues serialize only on the (otherwise idle) sync sequencer,
  * class-sum matmuls oh^T @ x on the PE in fp8 DoubleRow mode, one
    ldweights+2x512-col matmul group per chunk; PSUM S and T live in
    [16, 1024] two-bank tensors so each is evacuated with ONE copy,
  * sum-of-squares split per-tile across ACT (activation Square, accum_out,
    even tiles) and DVE (scalar_tensor_tensor x*x, accum_out, odd tiles),
    paced by chunk-arrival sems.  The ACT table load is the scalar engine's
    first instruction, fully hidden under the input stream,
  * outputs: one [16, 2048] f32 S|T DMA (sync) + one [128, 8] f32 accum DMA
    (gpsimd SWDGE); completion receipts hide inside the NEFF postamble's
    per-engine DRAIN + quiesce, so no out_sem wait.

The measured NEFF window is bounded below by ~10.4 us of runtime wrapper
(0.9 us bass preamble tail + ~9.5 us load-time injected postamble that
serially resets all 254 semaphores across the five engines); the kernel
body above targets the remaining ~8.5 us: stream 1.04 MB (~3-4 us,
overlapped), squares 4 tiles/engine (~5.6 us, the capacity wall), then
evac + output issues (~1.4 us tail).

fp8 notes: e4m3 quantization biases sum|x|^2 by ~+0.1% and the cross term
contributes only ~0.01% of the loss; measured end-to-end relative error
~7e-4, well inside the 2e-2 gate.  All accumulators (PSUM, accum_out) are
fp32.
"""

import numpy as np

N, D, NUM_CLASSES = 4096, 1024, 10
NCORES = 8
RPC = N // NCORES   # rows per core (both fm_s and fm_t are row-sharded)
KT = RPC // 128     # 128-row k-tiles per core per tensor (4)
W = 2 * KT          # total k-tiles per core (s then t) = 8
NCH = 4             # input chunks (2 tiles each)
CP = 16             # class dim padded for alignment
DW = D + CP         # tile width: data + appended one-hot columns

# square-op assignment: ACT takes even tiles, DVE odd — tile w arrives with
# chunk w//2, so both engines start after chunk 0 and stay chunk-paced.
SQ_ACT = [0, 2, 4, 6]
SQ_DVE = [1, 3, 5, 7]

_STATE = {}
LAST_RUN = None  # BassKernelResults of the most recent device run (for test.py)


def build_nc_raw():
    import concourse.bacc as bacc
    import concourse.mybir as mybir

    f32 = mybir.dt.float32
    f8 = mybir.dt.float8e4
    nc = bacc.Bacc(
        "TRN2",
        target_bir_lowering=False,
        debug=False,
        enable_asserts=False,
        num_devices=NCORES,
        # this kernel never reads the partition id (cores differ only by
        # their input slices) and uses no monotonic semaphores — dropping
        # both trims the framework preamble
        enable_partition_id=False,
        monotonic_sem_count=0,
    )

    bf16 = mybir.dt.bfloat16
    # chunk-major DRAM layout: chunk i is one fully contiguous 266 KB block
    # ([128, 2*DW]), so every partition's read is a single contiguous 2080 B
    # M2S descriptor — no strided DRAM scatter.
    x_in = nc.dram_tensor("x_in", (NCH, 128, 2 * DW), f8, kind="ExternalInput")
    sq_out = nc.dram_tensor("sq_out", (128, W), f32, kind="ExternalOutput")
    ST_out = nc.dram_tensor("ST_out", (CP, 2 * D), bf16, kind="ExternalOutput")

    x_sb = nc.alloc_sbuf_tensor("x_sb", [128, W, DW], f8)
    ST_sb = nc.alloc_sbuf_tensor("ST_sb", [CP, 2 * D], bf16)
    stats = nc.alloc_sbuf_tensor("stats", [128, W], f32)

    pS = nc.alloc_psum_tensor("pS", [CP, D], f32)  # 2 banks
    pT = nc.alloc_psum_tensor("pT", [CP, D], f32)  # 2 banks
    # fp8 square scratch: keeps the scratch WRITES small (1 KB/partition/op)
    # so they do not stall the input-DMA SBUF writes; each engine reuses its
    # own slot serially (engine program order makes that safe).  Only the
    # f32 accum_out feeds the result, the scratch value is never read.
    sq_scr = nc.alloc_sbuf_tensor("sq_scr", [128, 3, D], f8)

    k_sems = [nc.alloc_semaphore(f"k_sem{i}") for i in range(NCH)]
    sS = nc.alloc_semaphore("sS")
    sT = nc.alloc_semaphore("sT")
    ev = nc.alloc_semaphore("ev")
    sq_done = nc.alloc_semaphore("sq_done")
    out_sem = nc.alloc_semaphore("out_sem")

    Sq = mybir.ActivationFunctionType.Square
    ADD = mybir.AluOpType.add
    MUL = mybir.AluOpType.mult
    DR = mybir.MatmulPerfMode.DoubleRow

    xs = x_sb.ap()

    def square2(engine, wa, wb):
        # fused square + free-axis accumulate over TWO tiles in one op
        # (strided 2D AP skips the one-hot tails): stats[:, wa] = sum x^2.
        src = xs[:, wa : wb + 1 : wb - wa, 0:D]
        assert src.shape == (128, 2, D), src.shape
        if engine is nc.scalar:
            op = engine.activation(
                sq_scr.ap()[:, 0:2, :],
                src,
                Sq,
                accum_out=stats.ap()[:, wa : wa + 1],
            )
        else:
            op = engine.scalar_tensor_tensor(
                sq_scr.ap()[:, 0:2, :],
                src,
                0.0,
                src,
                ADD,
                MUL,
                accum_out=stats.ap()[:, wa : wa + 1],
            )
        op.then_inc(sq_done, 1)

    def square(engine, w):
        # fused square + free-axis accumulate: stats[:, w] = sum_d x^2 over
        # the 1024 data columns only (one-hot tail excluded).
        op = engine.scalar_tensor_tensor(
            sq_scr.ap()[:, 2, :],
            xs[:, w, 0:D],
            0.0,
            xs[:, w, 0:D],
            ADD,
            MUL,
            accum_out=stats.ap()[:, w : w + 1],
        )
        op.then_inc(sq_done, 1)

    with nc.Block() as block:

        @block.sync
        def _(sync):
            # chunks 0,1 on the sync HWDGE ring; 2,3 ride gpsimd's SWDGE
            # queue concurrently (independent descriptor path).
            for i in (0, 1):
                sync.dma_start(
                    x_sb.ap()[:, 2 * i : 2 * i + 2, :],
                    x_in.ap()[i, :, :],
                ).then_inc(k_sems[i], 16)
            # combined S|T output once both evacuations landed
            sync.wait_ge(ev, 2)
            sync.dma_start(ST_out.ap(), ST_sb.ap()).then_inc(out_sem, 16)
            # no explicit out_sem wait: the NEFF postamble's per-engine
            # DRAIN + runtime pending-DMA quiesce already order the output
            # DMAs before execution-complete
            sync.wait_ge(out_sem, 0)

        @block.scalar
        def _(scalar):
            # the ACT table load is auto-inserted before the first
            # activation — i.e. at t=0, fully hidden under the input stream.
            # Two-tile squares amortize the 352-cycle ACT op overhead; the
            # pairs (0,4) and (2,6) each combine an early and a late chunk.
            scalar.wait_ge(k_sems[0], 16)
            scalar.wait_ge(k_sems[2], 16)
            square2(scalar, 0, 4)
            scalar.wait_ge(k_sems[1], 16)
            scalar.wait_ge(k_sems[3], 16)
            square2(scalar, 2, 6)
            scalar.wait_ge(sS, 2)
            scalar.activation(
                ST_sb.ap()[:, 0:D], pS.ap(), mybir.ActivationFunctionType.Copy
            ).then_inc(ev, 1)

        @block.vector
        def _(vector):
            # per-tile squares in chunk-arrival order (c0, c2, c1, c3)
            for w in (1, 5, 3, 7):
                vector.wait_ge(k_sems[w // 2], 16)
                square(vector, w)
            vector.wait_ge(sT, 2)
            vector.tensor_copy(ST_sb.ap()[:, D : 2 * D], pT.ap()).then_inc(ev, 1)

        @block.gpsimd
        def _(gpsimd):
            for i in (2, 3):
                gpsimd.dma_start(
                    x_sb.ap()[:, 2 * i : 2 * i + 2, :],
                    x_in.ap()[i, :, :],
                ).then_inc(k_sems[i], 16)
            gpsimd.wait_ge(sq_done, 6)
            gpsimd.dma_start(sq_out.ap(), stats.ap()).then_inc(out_sem, 16)

        @block.tensor
        def _(tensor):
            # DoubleRow fp8: each matmul contracts a PAIR of 128-row k-tiles
            # (AP dim1 = pair index).  Accumulation groups per PSUM bank run
            # start-pair -> stop-pair, in chunk-arrival order (0,2,1,3).
            def mm(bank, dsem, pair, start, stop):
                a = 2 * pair
                lhsT = xs[:, a : a + 2, D:DW]
                for h in range(2):
                    m = tensor.matmul(
                        bank.ap()[:, 512 * h : 512 * (h + 1)],
                        lhsT,
                        xs[:, a : a + 2, 512 * h : 512 * (h + 1)],
                        start=start,
                        stop=stop,
                        perf_mode=DR,
                    )
                    if stop:
                        m.then_inc(dsem, 1)

            tensor.wait_ge(k_sems[0], 16)
            mm(pS, sS, 0, True, False)
            tensor.wait_ge(k_sems[2], 16)
            mm(pT, sT, 2, True, False)
            tensor.wait_ge(k_sems[1], 16)
            mm(pS, sS, 1, False, True)
            tensor.wait_ge(k_sems[3], 16)
            mm(pT, sT, 3, False, True)

    nc.compile()
    return nc


def _get_nc():
    if "nc" not in _STATE:
        _STATE["nc"] = build_nc_raw()
    return _STATE["nc"]


def kernel(fm_s, fm_t, targets, fusion_true=0, **_unused):
    global LAST_RUN
    import ml_dtypes
    from concourse.bass_utils import run_bass_kernel_spmd

    f8 = ml_dtypes.float8_e4m3
    fm_s = np.ascontiguousarray(np.asarray(fm_s, dtype=np.float32))
    fm_t = np.ascontiguousarray(np.asarray(fm_t, dtype=np.float32))
    tg = np.asarray(targets).astype(np.int64).ravel()
    assert fm_s.shape == (N, D) and fm_t.shape == (N, D) and tg.shape == (N,)

    oh = (tg[:, None] == np.arange(CP, dtype=np.int64)[None, :]).astype(np.float32)
    counts = np.bincount(tg, minlength=CP).astype(np.float64)[:CP]
    # append the one-hot columns to every row so each 128-row k-tile is
    # self-contained (the PE takes lhsT from the tile's own tail columns)
    s_aug = np.concatenate([fm_s, oh], axis=1).astype(f8)
    t_aug = np.concatenate([fm_t, oh], axis=1).astype(f8)

    in_maps = []
    for c in range(NCORES):
        # chunk-major + partition-major: chunk i holds tiles (2i, 2i+1);
        # x[i, p, :] = rows (256i + p, 256i + 128 + p) of this core's shard
        # (s tiles 0-3 then t tiles 4-7), each a contiguous 2080 B read.
        s_c = s_aug[c * RPC : (c + 1) * RPC].reshape(KT, 128, DW)
        t_c = t_aug[c * RPC : (c + 1) * RPC].reshape(KT, 128, DW)
        tiles = np.concatenate([s_c, t_c], axis=0)          # [W, 128, DW]
        x = np.ascontiguousarray(
            tiles.reshape(NCH, 2, 128, DW).transpose(0, 2, 1, 3).reshape(NCH, 128, 2 * DW)
        )
        in_maps.append({"x_in": x})

    nc = _get_nc()
    LAST_RUN = run_bass_kernel_spmd(nc, in_maps, list(range(NCORES)))
    res = LAST_RUN.results

    # stats slots actually written: ACT pairs -> cols 0 (tiles 0+4) and 2
    # (tiles 2+6); DVE per-tile -> cols 1, 3, 5, 7.  Cols 4, 6 are unused.
    used_cols = [0, 1, 2, 3, 5, 7]
    ss_tt = 0.0
    S = np.zeros((CP, D), np.float64)
    T = np.zeros((CP, D), np.float64)
    for r in res:
        ss_tt += float(r["sq_out"].astype(np.float64)[:, used_cols].sum())
        ST = r["ST_out"].astype(np.float64)
        S += ST[:, 0:D]
        T += ST[:, D : 2 * D]

    safe = np.where(counts > 0, counts, 1.0)
    dot = float(((S * T).sum(axis=1) / safe).sum())
    loss = (ss_tt - 2.0 * dot) / N
    return np.array(loss, dtype=np.float32)
